# revision 2
# baseline (speedup 1.0000x reference)
"""ArcFace loss distributed Bass kernel for 8 TRN2 NeuronCores.

Strategy (class-parallel over the 100000-class dim, fp8 compute):
  - Host: pad classes 100000 -> 8*12544, transpose W shard to [D, C_shard]
    per core, gather W[target] rows (pure data movement only).
  - Device (SPMD, identical program on 8 cores):
      * W is cast f32 -> fp8e4m3 INSIDE the DMA (SWDGE cast-DMA, exact RNE,
        verified on hw): raw |w| < 2^-7 lands subnormal, which the PE
        multiplies exactly; cosine is scale-invariant and the per-class
        norms come from the same quantized values, so quantization only
        adds ~1e-3 angle noise.  This removes the entire 42us ACT cast
        pass the f32-staging design needed.
      * x rows are normalized (*16), PE-transposed, cast to fp8 DoubleRow
        layout; cosines come from fp8 DR matmuls (stationary = W blocks,
        moving = xnT), 2 K-groups per 128-class block.
      * per-class ||w||^2 via fp8 DR gram matmuls interleaved into the
        LAST 40% of the previous tile's cos stream (the tile's cast-DMA
        has landed by then -- earlier placement head-of-line-blocks the
        in-order PE queue on the DMA wait); diag extracted on DVE,
        exp scales per tile on ACT (ln/exp on the pinned table set).
      * exp with fused -4.5 shift, fp8e4m3 out, split ACT native-exp /
        DVE Schraudolph (bitcast u8); class-sums via fp8 DR ones-matmuls
        with a 32-column stationary (se_ps is [32, B]; only row 0 is
        read) accumulating one PSUM bank across all 49 pairs, emitted one
        pair late so the PE never waits on the exp.
      * a dummy 2KB AllGather issued at t~13us warms the collective path
        (ncfw/SPAD setup + absorbs launch skew), cutting the real
        AllGather's trigger->start delay from ~11.5us to ~1.2us.
      * target-logit margin correction computed densely from host-gathered
        W[target] (exact f32), pad-class correction folded into it.
      * fused tail: AllGather [1,512] partial sums -> 8-row matmul-sum ->
        add margin delta -> ACT Ln with accum_out (sum of ln in one op) ->
        one STT -> scalar out.  Post-AG critical path ~7us.
Measured: 122-168us NEFF exec min-of-3 ~122-130 in aligned-skew windows
(vs 173us baseline); rel err 8.6e-04.  Remaining fixed costs: ~10us NEFF
preamble, ~78us HBM stream (25.7MB f32 at ~330GB/s), ~8us teardown,
cross-core launch skew absorbed at the AllGather.
"""

import math
from contextlib import ExitStack

import numpy as np

from concourse import bacc, masks, mybir, tile
from concourse.bass_utils import run_bass_kernel_spmd

N_CORES = 8
B = 512
D = 512
NCLASS = 100000
C_SHARD = NCLASS // N_CORES      # 12500
C_PAD = 12544                    # 98 * 128
S = 30.0
MARGIN = 0.5
COS_M = math.cos(MARGIN)
SIN_M = math.sin(MARGIN)
BIAS8 = 4.5
XSCALE = 16.0

# Schraudolph e4m3 exp: e^x ~ bitcast_e4m3(uint8(x*L2E*8 + 56 - C))
L2E = 1.4426950408889634
C_SCH = 486411.0 / 2.0 ** 20
B8_SCH = 56.0 - C_SCH - 8.0 * BIAS8 * L2E
_pv = int(round(B8_SCH))  # 4 -> subnormal 4/8 * 2^-6
V_PAD = (_pv / 8.0) * 2.0 ** -6 if _pv < 8 else (
    (1.0 + (_pv & 7) / 8.0) * 2.0 ** ((_pv >> 3) - 7))

f32 = mybir.dt.float32
bf16 = mybir.dt.bfloat16
u8 = mybir.dt.uint8
f8 = mybir.dt.float8e4
AF = mybir.ActivationFunctionType
ALU = mybir.AluOpType
AX = mybir.AxisListType
DR = mybir.MatmulPerfMode.DoubleRow

P = 128
TILE_SIZES = [256, 512, 1024] + [1792] * 6
N_DK = D // P                    # 4
N_D2 = N_DK // 2                 # 2
N_BK = B // P


def _pin_act_tables():
    import concourse.bacc as _bacc
    import concourse.hw_specs as _hw
    if getattr(_bacc, "_act_tables_pinned", False):
        return
    _orig = _hw.get_activation_tables

    def _pinned(arch):
        tabs = _orig(arch)
        both = {AF.Exp, AF.Ln}
        for name, fns in tabs.items():
            if name != "natural_log_exp_and_others":
                tabs[name] = fns - both
        return tabs

    _bacc.get_activation_tables = _pinned
    _bacc._act_tables_pinned = True


def _exp_on_dve(ck, n_blocks):
    """ACT carries ~70% of exps (it no longer does W casts); DVE takes the
    rest + diag extraction.  Last block fixed on DVE (V_PAD assumes the
    Schraudolph path for pad classes)."""
    if ck == n_blocks - 1:
        return True
    return ck % 10 >= 7


def build_arcface_nc(c_pad=C_PAD, c_real=C_SHARD, n_cores=N_CORES,
                     tile_sizes=None):
    if tile_sizes is None:
        tile_sizes = list(TILE_SIZES)
    assert sum(tile_sizes) == c_pad and all(t % (2 * P) == 0 for t in tile_sizes)
    n_tiles = len(tile_sizes)
    n_blocks = c_pad // P            # 98
    pad_corr = float(n_cores * (c_pad - c_real)) * V_PAD

    _pin_act_tables()
    nc = bacc.Bacc("TRN2", target_bir_lowering=False, debug=False,
                   num_devices=n_cores)

    wt_ext = nc.dram_tensor("wt", [D, c_pad], f32, kind="ExternalInput")
    x_ext = nc.dram_tensor("x", [B, D], f32, kind="ExternalInput")
    wtg_ext = nc.dram_tensor("wtg", [B, D], f32, kind="ExternalInput")
    out_ext = nc.dram_tensor("out", [1, 1], f32, kind="ExternalOutput")

    with ExitStack() as ctx:
        tc = ctx.enter_context(tile.TileContext(nc))
        cpool = ctx.enter_context(tc.tile_pool(name="consts", bufs=1))
        xpool = ctx.enter_context(tc.tile_pool(name="xpool", bufs=1))
        wpool = ctx.enter_context(tc.tile_pool(name="wpool", bufs=1))
        sm = ctx.enter_context(tc.tile_pool(name="smalls", bufs=1))
        spool = ctx.enter_context(tc.tile_pool(name="spool", bufs=3))
        epool = ctx.enter_context(tc.tile_pool(name="epool", bufs=8))
        jpool = ctx.enter_context(tc.tile_pool(name="jpool", bufs=3))
        ps_c = ctx.enter_context(tc.tile_pool(name="ps_c", bufs=4, space="PSUM"))
        ps_g = ctx.enter_context(tc.tile_pool(name="ps_g", bufs=3, space="PSUM"))
        ps_s = ctx.enter_context(tc.tile_pool(name="ps_s", bufs=1, space="PSUM"))
        dram = ctx.enter_context(tc.tile_pool(name="dram", bufs=1, space="DRAM"))

        # persistent fp8 W: [128, 4(d-subtile), c_pad], filled by cast-DMAs
        wall8 = wpool.tile([P, N_DK, c_pad], f8)
        w_src = wt_ext.ap().rearrange("(k p) c -> p k c", p=P)
        tile_c0 = np.cumsum([0] + tile_sizes).tolist()
        for t in range(3):
            c0, ct = tile_c0[t], tile_sizes[t]
            nc.gpsimd.dma_start(out=wall8[:, :, c0:c0 + ct],
                                in_=w_src[:, :, c0:c0 + ct])

        # ---- constants ----
        ident = cpool.tile([P, P], f32)
        masks.make_identity(nc, ident[:])
        ident_bf = cpool.tile([P, P], bf16)
        masks.make_identity(nc, ident_bf[:])
        ones_f = cpool.tile([P, 1], f32)
        nc.vector.memset(ones_f[:], 1.0)
        bias_mb = cpool.tile([P, 1], f32)
        nc.vector.memset(bias_mb[:], -BIAS8)
        ones8 = cpool.tile([P, 2, 32], f8)
        nc.vector.memset(ones8[:], 1.0)
        bias_lnA = cpool.tile([P, 1], f32)        # ln(S/XSCALE): ACT exp scale
        nc.vector.memset(bias_lnA[:], float(np.log(S / XSCALE)))
        bias_lnS = cpool.tile([P, 1], f32)        # ln(S*8*L2E/XSCALE): DVE
        nc.vector.memset(bias_lnS[:], float(np.log(S * 8.0 * L2E / XSCALE)))
        bias_lnX = cpool.tile([P, 1], f32)
        nc.vector.memset(bias_lnX[:], float(np.log(XSCALE)))
        warm = cpool.tile([P, 1], f32)
        nc.scalar.activation(warm[:], ones_f[:], AF.Ln)

        # dummy warm-up AllGather (2KB): absorbs the ~10us first-collective
        # setup cost while the main loop runs
        ccw_in = dram.tile([1, 4], f32)
        ccw_out = dram.tile([n_cores, 4], f32)
        warm_sb = sm.tile([1, 4], f32)
        nc.vector.memset(warm_sb[:], 1.0)
        nc.sync.dma_start(out=ccw_in[:], in_=warm_sb[:])
        nc.gpsimd.collective_compute(
            "AllGather", ALU.bypass,
            replica_groups=[list(range(n_cores))],
            ins=[ccw_in.opt()], outs=[ccw_out.opt()])
        for t in range(3, len(tile_sizes)):
            c0, ct = tile_c0[t], tile_sizes[t]
            nc.gpsimd.dma_start(out=wall8[:, :, c0:c0 + ct],
                                in_=w_src[:, :, c0:c0 + ct])

        # x: [B, D] -> [128, 4(b-block), D]
        xall = xpool.tile([P, N_BK, D], f32)
        nc.sync.dma_start(out=xall[:],
                          in_=x_ext.ap().rearrange("(k p) d -> p k d", k=N_BK))
        xb = [xall[:, k, :] for k in range(N_BK)]

        # running Esum accumulator bank
        se_ps = ps_s.tile([32, B], f32, name="se_ps")

        ssq_t = {}
        scA_t = {}
        scS_t = {}

        def blk_n(ck, d):
            """normal-mode [128,128] fp8 AP for class block ck, d-subtile d"""
            return wall8[:, d, ck * P:(ck + 1) * P]

        def blk_dr(ck, d2):
            """DR [128,2,128] AP for class block ck, K-group d2"""
            return wall8[:, 2 * d2:2 * d2 + 2, ck * P:(ck + 1) * P]

        def emit_gram_tile(t):
            ct = tile_sizes[t]
            n_sub = ct // P
            ssq = spool.tile([P, 16], f32, name="ssq")
            for s0 in range(0, n_sub, 2):
                ck0 = tile_c0[t] // P + s0
                gps = [ps_g.tile([P, P], f32, tag="g", name="g_ps")
                       for _ in range(2)]
                for d2 in range(N_D2):
                    for h in range(2):
                        bl = blk_dr(ck0 + h, d2)
                        nc.tensor.matmul(gps[h][:], bl, bl, start=(d2 == 0),
                                         stop=(d2 == N_D2 - 1), perf_mode=DR)
                for h in range(2):
                    junk_c = jpool.tile([P, P], f32, tag="junkg",
                                        name="junk_c")
                    nc.vector.scalar_tensor_tensor(
                        out=junk_c[:], in0=gps[h][:], scalar=1.0,
                        in1=ident[:], op0=ALU.mult, op1=ALU.mult,
                        accum_out=ssq[:, s0 + h:s0 + h + 1])
            ssq_t[t] = ssq

        def emit_scales(t):
            n_sub = tile_sizes[t] // P
            ssq = ssq_t.pop(t)
            # clamp on DVE (keeps the gpsimd queue free for cast-DMAs)
            nc.vector.tensor_scalar(out=ssq[:, :n_sub], in0=ssq[:, :n_sub],
                                    scalar1=1e-30, scalar2=None, op0=ALU.max)
            lnq = spool.tile([P, 16], f32, name="lnq")
            nc.scalar.activation(lnq[:, :n_sub], ssq[:, :n_sub], AF.Ln)
            scA = spool.tile([P, 16], f32, name="scA")
            nc.scalar.activation(scA[:, :n_sub], lnq[:, :n_sub], AF.Exp,
                                 bias=bias_lnA[:], scale=-0.5)
            scS = spool.tile([P, 16], f32, name="scS")
            nc.scalar.activation(scS[:, :n_sub], lnq[:, :n_sub], AF.Exp,
                                 bias=bias_lnS[:], scale=-0.5)
            scA_t[t] = scA
            scS_t[t] = scS

        n_pairs = n_blocks // 2
        pend_esum = []

        def flush_esum():
            while pend_esum:
                e2p, pi = pend_esum.pop(0)
                nc.tensor.matmul(se_ps[:], ones8[:], e2p[:],
                                 start=(pi == 0), stop=(pi == n_pairs - 1),
                                 perf_mode=DR)

        def emit_tile_cos(t, nxt):
            """cos+exp+esum for tile t; interleave gram/scales for tile nxt."""
            ct = tile_sizes[t]
            n_sub = ct // P
            ck_base = tile_c0[t] // P
            scA = scA_t.pop(t)
            scS = scS_t.pop(t)
            n_sub_nxt = (tile_sizes[nxt] // P) if nxt is not None else 0
            g_done = 0
            for s0 in range(0, n_sub, 2):
                cps = [ps_c.tile([P, B], f32, tag="cos", name="cos_ps")
                       for _ in range(2)]
                for d2 in range(N_D2):
                    for h in range(2):
                        nc.tensor.matmul(cps[h][:], blk_dr(ck_base + s0 + h, d2),
                                         xnt8[d2][:], start=(d2 == 0),
                                         stop=(d2 == N_D2 - 1), perf_mode=DR)
                while pend_esum:
                    e2p, pi = pend_esum.pop(0)
                    nc.tensor.matmul(se_ps[:], ones8[:], e2p[:],
                                     start=(pi == 0),
                                     stop=(pi == n_pairs - 1), perf_mode=DR)
                # interleave next tile's gram pairs into the LAST 40% of
                # the cos stream (tile nxt's cast-DMA has landed by then --
                # earlier placement head-of-line-blocks the PE queue)
                if nxt is not None:
                    done_frac = (s0 + 2) / n_sub
                    if done_frac <= 0.6:
                        g_goal = 0
                    else:
                        g_goal = min(n_sub_nxt, int(
                            n_sub_nxt * (done_frac - 0.6) / 0.4 + 0.999))
                    while g_done + 1 < g_goal:
                        ck0 = tile_c0[nxt] // P + g_done
                        gps = [ps_g.tile([P, P], f32, tag="g", name="g_ps")
                               for _ in range(2)]
                        for d2 in range(N_D2):
                            for h in range(2):
                                bl = blk_dr(ck0 + h, d2)
                                nc.tensor.matmul(gps[h][:], bl, bl,
                                                 start=(d2 == 0),
                                                 stop=(d2 == N_D2 - 1),
                                                 perf_mode=DR)
                        for h in range(2):
                            junk_c = jpool.tile([P, P], f32, tag="junkg",
                                                name="junk_c")
                            nc.vector.scalar_tensor_tensor(
                                out=junk_c[:], in0=gps[h][:], scalar=1.0,
                                in1=ident[:], op0=ALU.mult, op1=ALU.mult,
                                accum_out=ssq_t[nxt][:, g_done + h:g_done + h + 1])
                        g_done += 2
                e2_cur = epool.tile([P, 2, B], f8, tag="e", name="e2")
                for h in range(2):
                    s_i = s0 + h
                    ck = ck_base + s_i
                    e_sl = e2_cur[:, h, :]
                    if _exp_on_dve(ck, n_blocks):
                        nc.vector.tensor_scalar(
                            out=e_sl.bitcast(u8), in0=cps[h][:],
                            scalar1=scS[:, s_i:s_i + 1], scalar2=B8_SCH,
                            op0=ALU.mult, op1=ALU.add)
                    else:
                        nc.scalar.activation(e_sl, cps[h][:], AF.Exp,
                                             bias=bias_mb[:],
                                             scale=scA[:, s_i:s_i + 1])
                pend_esum.append((e2_cur, (ck_base + s0) // 2))
            if nxt is not None:
                while g_done + 1 < n_sub_nxt:
                    ck0 = tile_c0[nxt] // P + g_done
                    gps = [ps_g.tile([P, P], f32, tag="g", name="g_ps")
                           for _ in range(2)]
                    for d2 in range(N_D2):
                        for h in range(2):
                            bl = blk_dr(ck0 + h, d2)
                            nc.tensor.matmul(gps[h][:], bl, bl,
                                             start=(d2 == 0),
                                             stop=(d2 == N_D2 - 1),
                                             perf_mode=DR)
                    for h in range(2):
                        junk_c = jpool.tile([P, P], f32, tag="junkg",
                                            name="junk_c")
                        nc.vector.scalar_tensor_tensor(
                            out=junk_c[:], in0=gps[h][:], scalar=1.0,
                            in1=ident[:], op0=ALU.mult, op1=ALU.mult,
                            accum_out=ssq_t[nxt][:, g_done + h:g_done + h + 1])
                    g_done += 2
                emit_scales(nxt)

        # ---- x norms + normalize(*16) + transpose + fp8 cast ----
        qx = sm.tile([P, N_BK], f32)
        for k in range(N_BK):
            junk_a = jpool.tile([P, D], f32, tag="junk", name="junk_a")
            nc.vector.scalar_tensor_tensor(
                out=junk_a[:], in0=xb[k], scalar=1.0, in1=xb[k],
                op0=ALU.mult, op1=ALU.mult, accum_out=qx[:, k:k + 1])
        rx = sm.tile([P, N_BK], f32)
        nc.scalar.activation(rx[:], qx[:], AF.Ln)
        nc.scalar.activation(rx[:], rx[:], AF.Exp, bias=bias_lnX[:],
                             scale=-0.5)
        xn = [xpool.tile([P, D], bf16, name=f"xn{k}") for k in range(N_BK)]
        for k in range(N_BK):
            nc.vector.tensor_scalar(out=xn[k][:], in0=xb[k],
                                    scalar1=rx[:, k:k + 1], scalar2=None,
                                    op0=ALU.mult)
        xnt = [xpool.tile([P, B], bf16, name=f"xnt{d}") for d in range(N_DK)]
        for k in range(N_BK):
            tp_ps = ps_c.tile([P, B], bf16, tag="cos", name=f"tp_ps{k}")
            for d in range(N_DK):
                nc.tensor.transpose(tp_ps[:, d * P:(d + 1) * P],
                                    xn[k][:, d * P:(d + 1) * P], ident_bf[:])
            for d in range(N_DK):
                nc.vector.tensor_copy(xnt[d][:, k * P:(k + 1) * P],
                                      tp_ps[:, d * P:(d + 1) * P])
        xnt8 = [xpool.tile([P, 2, B], f8, name=f"xnt8_{d2}")
                for d2 in range(N_D2)]
        for d in range(N_DK):
            d2, j = divmod(d, 2)
            nc.scalar.activation(xnt8[d2][:, j, :], xnt[d][:], AF.Copy)

        # tile 0's gram/scales run standalone before the first cos
        emit_gram_tile(0)
        emit_scales(0)

        # ---- main loop ----
        for t in range(n_tiles):
            if t + 1 < n_tiles:
                ssq_t[t + 1] = spool.tile([P, 16], f32, name="ssq")
                emit_tile_cos(t, t + 1)
            else:
                emit_tile_cos(t, None)
        flush_esum()

        # ---- target margin terms (dense over all B rows, every core) ----
        wgall = xpool.tile([P, N_BK, D], f32)
        nc.sync.dma_start(
            out=wgall[:],
            in_=wtg_ext.ap().rearrange("(k p) d -> p k d", k=N_BK))
        qw = sm.tile([P, N_BK], f32)
        pt = sm.tile([P, N_BK], f32)
        for k in range(N_BK):
            junk_b = jpool.tile([P, D], f32, tag="junk", name="junk_b")
            nc.vector.scalar_tensor_tensor(
                out=junk_b[:], in0=wgall[:, k, :], scalar=1.0,
                in1=wgall[:, k, :],
                op0=ALU.mult, op1=ALU.mult, accum_out=qw[:, k:k + 1])
            junk_d = jpool.tile([P, D], f32, tag="junk", name="junk_d")
            nc.vector.scalar_tensor_tensor(
                out=junk_d[:], in0=xb[k], scalar=1.0,
                in1=wgall[:, k, :],
                op0=ALU.mult, op1=ALU.mult, accum_out=pt[:, k:k + 1])
        q = sm.tile([P, N_BK], f32)
        nc.vector.tensor_mul(q[:], qw[:], qx[:])
        nc.vector.tensor_scalar(out=q[:], in0=q[:], scalar1=1e-30,
                                scalar2=None, op0=ALU.max)
        rq = sm.tile([P, N_BK], f32)
        nc.scalar.activation(rq[:], q[:], AF.Ln)
        nc.scalar.activation(rq[:], rq[:], AF.Exp, scale=-0.5)
        cos_t = sm.tile([P, N_BK], f32)
        nc.vector.tensor_mul(cos_t[:], pt[:], rq[:])
        om = sm.tile([P, N_BK], f32)
        nc.vector.tensor_mul(om[:], cos_t[:], cos_t[:])
        nc.vector.tensor_scalar(out=om[:], in0=om[:], scalar1=-1.0,
                                scalar2=1.0, op0=ALU.mult, op1=ALU.add)
        nc.vector.tensor_scalar(out=om[:], in0=om[:], scalar1=1e-36,
                                scalar2=None, op0=ALU.max)
        sine = sm.tile([P, N_BK], f32)
        nc.scalar.activation(sine[:], om[:], AF.Ln)
        nc.scalar.activation(sine[:], sine[:], AF.Exp, scale=0.5)
        tmp = sm.tile([P, N_BK], f32)
        nc.vector.tensor_scalar(out=tmp[:], in0=cos_t[:], scalar1=COS_M,
                                scalar2=None, op0=ALU.mult)
        phi = sm.tile([P, N_BK], f32)
        nc.vector.scalar_tensor_tensor(out=phi[:], in0=sine[:], scalar=-SIN_M,
                                       in1=tmp[:], op0=ALU.mult, op1=ALU.add)
        mask = sm.tile([P, N_BK], mybir.dt.uint8)
        nc.vector.tensor_scalar(out=mask[:], in0=cos_t[:], scalar1=0.0,
                                scalar2=None, op0=ALU.is_gt)
        phi_f = sm.tile([P, N_BK], f32)
        nc.vector.select(phi_f[:], mask[:], phi[:], cos_t[:])
        # delta = exp(S*phi_f - B) - exp(S*cos_t - B) - pad_corr/... folded:
        e1 = sm.tile([P, N_BK], f32)
        nc.scalar.activation(e1[:], phi_f[:], AF.Exp, bias=bias_mb[:], scale=S)
        e2 = sm.tile([P, N_BK], f32)
        nc.scalar.activation(e2[:], cos_t[:], AF.Exp, bias=bias_mb[:], scale=S)
        delta = sm.tile([P, N_BK], f32)
        # delta = (e1 - pad_corr) - e2   (pad_corr folded here, per element)
        nc.vector.scalar_tensor_tensor(out=delta[:], in0=e1[:],
                                       scalar=-pad_corr, in1=e2[:],
                                       op0=ALU.add, op1=ALU.subtract)
        # flip delta/phi_f to [1, B] row layout
        dp_ps = ps_g.tile([1, B], f32, tag="g", name="dp_ps")
        pp_ps = ps_g.tile([1, B], f32, tag="g", name="pp_ps")
        for k in range(N_BK):
            nc.tensor.transpose(dp_ps[0:1, k * P:(k + 1) * P],
                                delta[:, k:k + 1], ident[:])
            nc.tensor.transpose(pp_ps[0:1, k * P:(k + 1) * P],
                                phi_f[:, k:k + 1], ident[:])
        delta_row = sm.tile([1, B], f32)
        nc.vector.tensor_copy(delta_row[:], dp_ps[:])
        phi_row = sm.tile([1, B], f32)
        nc.vector.tensor_copy(phi_row[:], pp_ps[:])
        # tail constant: BIAS8 - (S/B) * sum(phi)
        phisum = sm.tile([1, 1], f32)
        nc.vector.reduce_sum(phisum[:], phi_row[:], axis=AX.X)
        tailc = sm.tile([1, 1], f32)
        nc.vector.tensor_scalar(out=tailc[:], in0=phisum[:],
                                scalar1=-S / float(B), scalar2=BIAS8,
                                op0=ALU.mult, op1=ALU.add)

        # ---- collective: AllGather partial sums ----
        sumE_sb = sm.tile([1, B], f32)
        nc.vector.tensor_copy(sumE_sb[:], se_ps[0:1, :])
        cc_in = dram.tile([1, B], f32)
        cc_out = dram.tile([n_cores, B], f32)
        nc.sync.dma_start(out=cc_in[:], in_=sumE_sb[:])
        nc.gpsimd.collective_compute(
            "AllGather", ALU.bypass,
            replica_groups=[list(range(n_cores))],
            ins=[cc_in.opt()], outs=[cc_out.opt()])
        red = sm.tile([n_cores, B], f32)
        nc.sync.dma_start(out=red[:], in_=cc_out[:])
        tot_ps = ps_g.tile([1, B], f32, tag="g", name="tot_ps")
        nc.tensor.matmul(tot_ps[:], ones_f[0:n_cores, :], red[:],
                         start=True, stop=True)

        # ---- fused tail: out = tailc + (1/B) * sum_b ln(total_b) ----
        total = sm.tile([1, B], f32)
        nc.vector.tensor_add(total[:], tot_ps[:], delta_row[:])
        lnt = sm.tile([1, B], f32)
        lnacc = sm.tile([1, 1], f32)
        nc.scalar.activation(lnt[:], total[:], AF.Ln, accum_out=lnacc[:])
        mean_sb = sm.tile([1, 1], f32)
        nc.vector.scalar_tensor_tensor(out=mean_sb[:], in0=lnacc[:],
                                       scalar=1.0 / float(B), in1=tailc[:],
                                       op0=ALU.mult, op1=ALU.add)
        nc.sync.dma_start(out=out_ext.ap()[:, :], in_=mean_sb[:])

    nc.compile()
    return nc


def _shard_inputs(input, weight, target, c_pad=C_PAD, c_real=C_SHARD,
                  n_cores=N_CORES):
    """Host-side data layout only: shard, pad, transpose, gather."""
    x = np.ascontiguousarray(input, dtype=np.float32)
    w = np.asarray(weight, dtype=np.float32)
    tgt = np.asarray(target).astype(np.int64)
    wtg = np.ascontiguousarray(w[tgt])  # [B, D] gathered target rows
    in_maps = []
    for j in range(n_cores):
        shard = w[j * c_real:(j + 1) * c_real]          # [c_real, D]
        wt = np.zeros((D, c_pad), dtype=np.float32)
        wt[:, :c_real] = shard.T
        in_maps.append({"wt": wt, "x": x, "wtg": wtg})
    return in_maps


_NC_CACHE = {}


def kernel(input, weight, target, _trace=False, _trace_kwargs=None):
    key = "full"
    if key not in _NC_CACHE:
        _NC_CACHE[key] = build_arcface_nc()
    nc = _NC_CACHE[key]
    in_maps = _shard_inputs(input, weight, target)
    res = run_bass_kernel_spmd(nc, in_maps, core_ids=list(range(N_CORES)),
                               trace=_trace, **(_trace_kwargs or {}))
    out = np.float32(res.results[0]["out"][0, 0])
    kernel.last_results = res
    return np.asarray(out, dtype=np.float32).reshape(())


if __name__ == "__main__":
    rng = np.random.default_rng(0)
    x = rng.standard_normal((B, D)).astype(np.float32)
    w = rng.standard_normal((NCLASS, D)).astype(np.float32) * 0.01
    t = rng.integers(0, NCLASS, size=(B,)).astype(np.int64)
    print("out:", kernel(x, w, t))
